# revision 18
# baseline (speedup 1.0000x reference)
"""nn_ConvP4 TRN2 Bass kernel v4: mixed bf16 / fp8-DoubleRow taps.

P4 group-equivariant conv: out[b,j] = sum_k x[b, j+k-1] * rot90(kernel, j)[:,:,k]
  x: [8, 4, 128, 128, 64] f32, kernel: [3,3,3,64,128] f32 -> out [8,4,128,128,128].

Data-parallel over batch (one b per NeuronCore). The PE column rate
(1 col/cycle, ~273ns per N=512 matmul at the 8-core power envelope) makes
matmul COUNT the only lever: a DoubleRow fp8 matmul contracts K_eff=256
(4 taps x 64ch) in the same wall time as a bf16 K=128 matmul (2 taps).

Tap plan per output chain (27 taps):
  bf16 slots 0-8: (a,v) pairs, low=plane j-1 (k=0), high=plane j (k=1) [P0]
  k=2 plane (9 taps):
    plan-B (sb 3, 13 MMs): bf16 slots 9-11 [P1 row-shift pairs]
      + 1 fp8 DR matmul covering taps (2,0),(2,1),(2,2) [P3]
    plan-A (sb 0-2, 12 MMs): 1 fp8 DR matmul for taps
      (0,0),(1,0),(0,1),(1,1) [P4] + bf16 slot 11 [P1] + DR [P3]
  DR3's otherwise-idle (i=1, high) quarter carries the fp8 residual
  xl = x - fp8(x) of tap (2,2), restoring near-bf16 accuracy there.
  Pure-fp8 taps carry ~0.038 rel err; the 3:1 plan-A:plan-B mix gives
  global rel err 0.0172 (HW-verified; gate 2e-2), simulated exactly on
  the seed-0 inputs beforehand (sim-HW agreement ~1e-4 absolute).
  Measured ~423-431us vs the 489us all-bf16 baseline (12.25 vs 14
  matmuls per chain; the PE column rate at the 8-core power envelope
  is the binding resource).

Also: tile_legalize emits a Ldweights before every matmul; with the
slot-outer/q-inner order each weight is reused by 8 consecutive matmuls,
and _strip_redundant_ldweights deletes the duplicate loads in BIR
(walrus supports non-self-loading bf16/fp8 matmuls).
"""

import json
from contextlib import ExitStack

import numpy as np
import ml_dtypes

import bass_rust
import concourse.bacc as bacc
import concourse.tile as tile
from concourse import mybir
from concourse.bass_utils import run_bass_kernel_spmd

N_CORES = 8
B = 8
H = W = 128
CIN = 64
F = 128
HP = 132
WP = 130    # bf16 window cols
WPP = 136   # fp8 window cols (pad so 34*WPP % 16 == 0 for the DR i-stride)
SBS = 32          # superblock rows
NSB = H // SBS    # 4 superblocks per j
NQ = SBS // 4     # 8 psum chains per superblock
NBF = 12          # bf16 weight slots per j (0-8 P0, 9-11 P1)

F32 = mybir.dt.float32
BF16 = mybir.dt.bfloat16
FP8 = mybir.dt.float8e4
BF = ml_dtypes.bfloat16
E4 = ml_dtypes.float8_e4m3fn


def _wkey(inst):
    return json.dumps(
        [inst.get("ins"), inst.get("tile_position"), inst.get("tile_size"),
         inst.get("perf_mode"), inst.get("is_transpose")],
        sort_keys=True,
    )


def _strip_redundant_ldweights(nc):
    """Delete Ldweights whose weights AP matches the already-loaded state;
    merge their semaphore waits/updates into the next kept PE instruction."""
    j = json.loads(bass_rust.module_to_json_string(nc.m))
    for f in j["functions"]:
        for blk in f["blocks"]:
            insts = blk.get("instructions", [])
            out = []
            cur_w = None
            pend_wait, pend_upd = [], []
            for inst in insts:
                op = inst.get("opcode")
                if op == "Ldweights":
                    key = _wkey(inst)
                    si = inst.get("sync_info") or {}
                    if key == cur_w:
                        pend_wait.extend(si.get("on_wait") or [])
                        pend_upd.extend(si.get("on_update") or [])
                        continue
                    cur_w = key
                elif inst.get("engine") == "PE" and op != "Matmult":
                    cur_w = None
                if (pend_wait or pend_upd) and inst.get("engine") == "PE":
                    si = inst.setdefault(
                        "sync_info", {"on_wait": [], "on_update": []}
                    )
                    si.setdefault("on_wait", []).extend(pend_wait)
                    si.setdefault("on_update", []).extend(pend_upd)
                    pend_wait, pend_upd = [], []
                out.append(inst)
            assert not pend_wait and not pend_upd
            blk["instructions"] = out
    nc.m = bass_rust.module_from_json_string(json.dumps(j))
    return nc


def _stage_inputs(x: np.ndarray, kern: np.ndarray):
    xt = np.ascontiguousarray(x.transpose(0, 1, 4, 2, 3))  # [b,g,c,h,w]
    xb = xt.astype(BF)
    x8 = np.clip(xt, -240, 240).astype(E4)
    xl8 = (xt - x8.astype(np.float32)).astype(E4)  # fp8 residual of x

    P0 = np.zeros((B, 4, 128, HP, WP), BF)
    P1 = np.zeros((B, 4, 128, HP, WP), BF)
    for t in range(4):
        P0[:, t, 0:64, 1 : H + 1, 1 : W + 1] = xb[:, t]
        P0[:, t, 64:128, 1 : H + 1, 1 : W + 1] = xb[:, (t + 1) % 4]
        P1[:, t, 0:64, 2 : H + 2, 1 : W + 1] = xb[:, t]
        P1[:, t, 64:128, 1 : H + 1, 1 : W + 1] = xb[:, t]

    # fp8 DR windows, layout [t, ch(part), i(pair), R, C]. Delivered value at
    # view [row r+1, col c+o] must be the tap input x[r+a-1, c+v-1]; staged
    # offsets bake the shifts.
    P3 = np.zeros((B, 4, 128, 2, HP, WPP), E4)
    P4 = np.zeros((B, 4, 128, 2, HP, WPP), E4)
    for t in range(4):
        # DR3 view cols 1:1+W  -> taps (2,0),(2,1),(2,2) of plane t
        P3[:, t, 0:64, 0, 2 : H + 2, 2 : W + 2] = x8[:, t]
        P3[:, t, 64:128, 0, 2 : H + 2, 1 : W + 1] = x8[:, t]
        P3[:, t, 0:64, 1, 2 : H + 2, 0:W] = x8[:, t]
        # i=1 high half: xl residual of tap (2,2) -> near-bf16 accuracy for
        # that tap, using the otherwise-idle quarter of the DR3 matmul
        P3[:, t, 64:128, 1, 2 : H + 2, 0:W] = xl8[:, t]
        # DR4 view cols 0:W    -> taps (0,0),(1,0),(0,1),(1,1) of plane t
        P4[:, t, 0:64, 0, 2 : H + 2, 1 : W + 1] = x8[:, t]
        P4[:, t, 64:128, 0, 1 : H + 1, 1 : W + 1] = x8[:, t]
        P4[:, t, 0:64, 1, 2 : H + 2, 0:W] = x8[:, t]
        P4[:, t, 64:128, 1, 1 : H + 1, 0:W] = x8[:, t]

    Wpk = np.zeros((4, NBF, 128, F), np.float32)
    Wdr = np.zeros((4, 2, 128, 2, F), np.float32)
    for j in range(4):
        Kj = np.rot90(kern, k=j, axes=(0, 1))
        for a in range(3):
            for v in range(3):
                Wpk[j, 3 * a + v, 0:64] = Kj[a, v, 0]
                Wpk[j, 3 * a + v, 64:128] = Kj[a, v, 1]
        for v in range(3):
            Wpk[j, 9 + v, 0:64] = Kj[0, v, 2]
            Wpk[j, 9 + v, 64:128] = Kj[1, v, 2]
        # DR set 0 (P3): i=0 -> (2,0)|(2,1), i=1 -> (2,2) xh | (2,2) xl
        Wdr[j, 0, 0:64, 0] = Kj[2, 0, 2]
        Wdr[j, 0, 64:128, 0] = Kj[2, 1, 2]
        Wdr[j, 0, 0:64, 1] = Kj[2, 2, 2]
        Wdr[j, 0, 64:128, 1] = Kj[2, 2, 2]
        # DR set 1 (P4): i=0 -> (0,0)|(1,0), i=1 -> (0,1)|(1,1)
        Wdr[j, 1, 0:64, 0] = Kj[0, 0, 2]
        Wdr[j, 1, 64:128, 0] = Kj[1, 0, 2]
        Wdr[j, 1, 0:64, 1] = Kj[0, 1, 2]
        Wdr[j, 1, 64:128, 1] = Kj[1, 1, 2]
    Wb = Wpk.astype(BF)
    W8 = np.clip(Wdr, -240, 240).astype(E4)
    return [
        {"p0": P0[b], "p1": P1[b], "p3": P3[b], "p4": P4[b],
         "wt": Wb, "wdr": W8}
        for b in range(B)
    ]


def build_program(loop_iters: int = 1, out_bufs: int = 8, win_bufs: int = 3,
                  strip: bool = True):
    nc = bacc.Bacc("TRN2", target_bir_lowering=False, debug=False, num_devices=N_CORES)

    p0 = nc.dram_tensor("p0", [4, 128, HP, WP], BF16, kind="ExternalInput").ap()
    p1 = nc.dram_tensor("p1", [4, 128, HP, WP], BF16, kind="ExternalInput").ap()
    p3 = nc.dram_tensor("p3", [4, 128, 2, HP, WPP], FP8, kind="ExternalInput").ap()
    p4 = nc.dram_tensor("p4", [4, 128, 2, HP, WPP], FP8, kind="ExternalInput").ap()
    wt = nc.dram_tensor("wt", [4, NBF, 128, F], BF16, kind="ExternalInput").ap()
    wdr = nc.dram_tensor("wdr", [4, 2, 128, 2, F], FP8, kind="ExternalInput").ap()
    out = nc.dram_tensor("out_t", [4, F, H, W], F32, kind="ExternalOutput").ap()

    win_rows = SBS + 2  # 34

    with tile.TileContext(nc) as tc, ExitStack() as ctx:
        wpool = ctx.enter_context(tc.tile_pool(name="wts", bufs=1))
        winpool = ctx.enter_context(tc.tile_pool(name="win", bufs=win_bufs))
        pspool = ctx.enter_context(tc.tile_pool(name="ps", bufs=8, space="PSUM"))
        outpool = ctx.enter_context(tc.tile_pool(name="ob", bufs=out_bufs))

        w_all = wpool.tile([128, 4 * NBF * F], BF16)
        nc.sync.dma_start(
            w_all[:].rearrange("p (s f) -> p s f", f=F),
            wt.rearrange("j s c f -> c (j s) f"),
        )
        wdr_all = wpool.tile([128, 4 * 2 * 2 * F], FP8)
        nc.sync.dma_start(
            wdr_all[:].rearrange("p (s i f) -> p s i f", i=2, f=F),
            wdr.rearrange("j s c i f -> c (j s) i f"),
        )

        def body(_iv=None):
            for j in range(4):
                t0 = (j + 3) % 4
                t1 = (j + 1) % 4
                for sb in range(NSB):
                    plan_a = (sb != 3)  # alpha = 0.75
                    h0 = SBS * sb
                    win0 = winpool.tile([128, win_rows * WP], BF16, tag="win0")
                    nc.sync.dma_start(
                        win0[:].rearrange("p (r c) -> p r c", c=WP),
                        p0[t0, :, h0 : h0 + win_rows, :],
                    )
                    win1 = winpool.tile([128, win_rows * WP], BF16, tag="win1")
                    nc.scalar.dma_start(
                        win1[:].rearrange("p (r c) -> p r c", c=WP),
                        p1[t1, :, h0 + 1 : h0 + 1 + win_rows, :],
                    )
                    win3 = winpool.tile(
                        [128, 2 * win_rows * WPP], FP8, tag="win3"
                    )
                    nc.gpsimd.dma_start(
                        win3[:].rearrange(
                            "p (i r c) -> p i r c", i=2, c=WPP
                        ),
                        p3[t1, :, :, h0 + 1 : h0 + 1 + win_rows, :],
                    )
                    if plan_a:
                        win4 = winpool.tile(
                            [128, 2 * win_rows * WPP], FP8, tag="win4"
                        )
                        nc.sync.dma_start(
                            win4[:].rearrange(
                                "p (i r c) -> p i r c", i=2, c=WPP
                            ),
                            p4[t1, :, :, h0 + 1 : h0 + 1 + win_rows, :],
                        )
                    w0r = win0[:].rearrange("p (r c) -> p r c", c=WP)
                    w1r = win1[:].rearrange("p (r c) -> p r c", c=WP)
                    w3r = win3[:].rearrange("p (i r c) -> p i r c", i=2, c=WPP)
                    if plan_a:
                        w4r = win4[:].rearrange(
                            "p (i r c) -> p i r c", i=2, c=WPP
                        )

                    psums = [
                        pspool.tile([128, 512], F32, tag="ps",
                                    name=f"ps_{j}_{sb}_{q}")
                        for q in range(NQ)
                    ]
                    # slot schedule: list of (kind, payload)
                    slots = [("bf", s) for s in range(9)]
                    if plan_a:
                        slots += [("dr4", None), ("bf", 11), ("dr3", None)]
                    else:
                        slots += [("bf", 9), ("bf", 10), ("bf", 11),
                                  ("dr3", None)]
                    n_s = len(slots)
                    for si, (kind, s) in enumerate(slots):
                        for q in range(NQ):
                            r0 = 4 * q
                            if kind == "bf":
                                sl = (j * NBF + s) * F
                                lhsT = w_all[:, sl : sl + F]
                                if s < 9:
                                    a, v = divmod(s, 3)
                                    rhs = w0r[:, r0 + a : r0 + a + 4,
                                              v : v + W]
                                else:
                                    v = s - 9
                                    rhs = w1r[:, r0 : r0 + 4, v : v + W]
                                pm = None
                            else:
                                dset = 0 if kind == "dr3" else 1
                                sl = (j * 2 + dset) * 2 * F
                                lhsT = wdr_all[
                                    :, sl : sl + 2 * F
                                ].rearrange("p (i f) -> p i f", i=2)
                                if kind == "dr3":
                                    rhs = w3r[:, :, r0 + 2 : r0 + 6,
                                              1 : 1 + W]
                                else:
                                    rhs = w4r[:, :, r0 : r0 + 4, 0:W]
                                pm = mybir.MatmulPerfMode.DoubleRow
                            nc.tensor.matmul(
                                psums[q][:],
                                lhsT,
                                rhs,
                                start=(si == 0),
                                stop=(si == n_s - 1),
                                perf_mode=pm,
                            )
                    for q in range(NQ):
                        ot = outpool.tile([128, 512], F32, tag="ob")
                        if q % 2 == 0:
                            nc.vector.tensor_copy(ot[:], psums[q][:])
                        else:
                            nc.scalar.copy(ot[:], psums[q][:])
                        dq = (nc.sync, nc.scalar, nc.gpsimd)[q % 3]
                        dq.dma_start(
                            out[j, :, h0 + 4 * q : h0 + 4 * q + 4, :],
                            ot[:].rearrange("p (r c) -> p r c", c=W),
                        )

        if loop_iters > 1:
            with tc.For_i(0, loop_iters, 1) as iv:
                body(iv)
        else:
            body()

    nc.compile()
    if strip:
        _strip_redundant_ldweights(nc)
    return nc


_PROGRAM_CACHE = {}


def _get_program(loop_iters: int = 1):
    if loop_iters not in _PROGRAM_CACHE:
        _PROGRAM_CACHE[loop_iters] = build_program(loop_iters)
    return _PROGRAM_CACHE[loop_iters]


def kernel(**inputs) -> np.ndarray:
    x = np.ascontiguousarray(np.asarray(inputs["x"], dtype=np.float32))
    kern = np.ascontiguousarray(np.asarray(inputs["kernel"], dtype=np.float32))
    assert x.shape == (B, 4, H, W, CIN), x.shape
    assert kern.shape == (3, 3, 3, CIN, F), kern.shape

    nc = _get_program(1)
    in_maps = _stage_inputs(x, kern)

    last_err = None
    for _attempt in range(3):
        try:
            res = run_bass_kernel_spmd(nc, in_maps, list(range(N_CORES)))
            break
        except Exception as e:
            last_err = e
    else:
        raise last_err

    stacked = np.stack([r["out_t"] for r in res.results])  # [b, j, f, h, w]
    return np.ascontiguousarray(stacked.transpose(0, 1, 3, 4, 2))
